# revision 44
# baseline (speedup 1.0000x reference)
"""Trainium2 Bass kernel for GQA multi-head attention with RoPE.

Problem: B=2, T=2048, C=2048, 16 q-heads, 4 kv-heads, HD=128, causal, RoPE.

Sharding (8 cores): tensor-parallel over the 4 kv-head groups x data-parallel
over the 2 batch elements. Core c handles batch c//4, kv-group c%4 (4 q-heads).
Each core computes x @ wq/wk/wv for its head group, RoPE, causal attention,
and a partial output projection (rows of wo for its heads). The host sums the
4 partial fp16 outputs per batch element.

Numerics: fp16 operands everywhere (PE runs fp16 at 1.0 cycles/row for any
free size, and 2-byte dtypes unlock the DVE 2x/4x fast modes), f32 psum
accumulation, fp16 output (host upcasts before the partial-sum).  Softmax
skips the max-subtraction (scores are bounded here); the causal mask is a
-1e5 additive bias on diagonal blocks, fully-masked tiles are skipped, and
diagonal-column tiles are trimmed to their live span.

Schedule highlights (each measured on neuron-profile traces):
- weight stream rides gpsimd+scalar DMA channels in ct consumption order;
  the sync channel carries only the x stream, so phase-1 chunk 0 starts
  ~3us earlier and streams without weight stalls.
- softmax denominator: exp tiles are tree-folded on the DVE (fp16 fast
  mode) to one full tile + one staggered diag tile, so the PE pays only 2
  ones-matmuls per (head, chunk) instead of up to 8.
- wo projection groups of attention chunk i are drip-fed between the
  emission steps of attention chunk i+1: their matmuls fill the PE bubbles
  left by the den->recip->broadcast latency, and their psum-drain copies
  never queue ahead of chunk i+1's exps on Scalar/Vector.
- the final chunk's wo groups run after the attention psum pools close,
  in a dedicated 6-bank psum pool, so the drain never blocks rotation.
- adjacent output tiles pair into one [128,1024] fp16 store (half the DMA
  issues); psum->sbuf drains are split scalar/vector halves.

Dead ends (measured worse, do not revisit): computing q/k transposed
directly from flipped matmuls (RoPE needs cross-partition combines the
DVE cannot do); pairing score tiles into 2-bank psum for one shared exp
(coarser pssc rotation loses more than the Act overhead saved); taking a
pswo bank from pssc or psden; expt bufs=3; deeper xt/outbuf pools.
"""

import sys

sys.path.insert(0, "/opt/trn_rl_repo")

import numpy as np

B, T, C = 2, 2048, 2048
N_KV = 4
G = 4           # q heads per kv head
HD = 128
NCORES = 8
TT = T // 128   # 16 t-tiles
CT = C // 128   # 16 c-tiles
NTC = 4         # 512-wide t chunks
SCALE = float(1.0 / np.sqrt(HD))
MASK_BIAS = -1.0e5

_CACHE = {}
LAST_RESULTS = None


def _build():
    import concourse.bass as bass
    import concourse.tile as tile
    from concourse import mybir, bacc

    def bcast_mid(ap2d, reps):
        """[128, N] AP -> [128, reps, N] with a stride-0 middle dim."""
        return bass.AP(tensor=ap2d.tensor, offset=ap2d.offset,
                       ap=[list(ap2d.ap[0]), [0, reps], list(ap2d.ap[1])])

    f32, f32r, f16 = mybir.dt.float32, mybir.dt.float32r, mybir.dt.float16

    nc = bacc.Bacc()
    xT = nc.dram_tensor("xT", [128, 8 * 4 * 4 * 256], f16, kind="ExternalInput")
    wqkv = nc.dram_tensor("wqkv", [128, CT * 768], f16, kind="ExternalInput")
    wo = nc.dram_tensor("wo", [128, G * C], f16, kind="ExternalInput")
    fcos = nc.dram_tensor("fcos", [128, TT * 64], f16, kind="ExternalInput")
    fsin = nc.dram_tensor("fsin", [128, TT * 64], f16, kind="ExternalInput")
    cident = nc.dram_tensor("cident", [128, 128], f16, kind="ExternalInput")
    cones = nc.dram_tensor("cones", [128, 1], f16, kind="ExternalInput")
    ctri = nc.dram_tensor("ctri", [128, 128], f32, kind="ExternalInput")
    out = nc.dram_tensor("out", [T, C], f16, kind="ExternalOutput")

    with tile.TileContext(nc) as tc:
        with (
            tc.tile_pool(name="consts", bufs=1) as cpool,
            tc.tile_pool(name="persist", bufs=1) as ppool,
            tc.tile_pool(name="outbuf", bufs=4) as opool,
        ):
            ident_sb = cpool.tile([128, 128], f16)
            ones_sb = cpool.tile([128, 1], f16)
            tri_sb = cpool.tile([128, 128], f32)

            # ---- persistent activations ----
            qT_sb = ppool.tile([128, G, T], f16)      # [d, h, t]
            kT_sb = ppool.tile([128, T], f16)         # [d, s]
            v_sb = ppool.tile([128, TT, HD], f16)     # [s%128, s//128, d]
            outT_sb = ppool.tile([128, G, T], f16)    # [d, h, t]

            # ================= Phase 1: QKV projection + RoPE + transpose ====
            with (
                tc.tile_pool(name="weights", bufs=1) as wpool,
                tc.tile_pool(name="freqs", bufs=1) as fpool,
                tc.tile_pool(name="xt", bufs=4) as xtp,
                tc.tile_pool(name="ropet", bufs=8) as rtp,
                tc.tile_pool(name="qr", bufs=4) as qrp,
                tc.tile_pool(name="kr", bufs=4) as krp,
                tc.tile_pool(name="ppq", bufs=3, space="PSUM") as ppq,
                tc.tile_pool(name="ppkv", bufs=3, space="PSUM") as ppkv,
                tc.tile_pool(name="pptr", bufs=2, space="PSUM") as pptr,
            ):
                wsrc = cpool.tile([128, 128], f16)
                nc.gpsimd.memset(wsrc[:], 0.0)
                nc.sync.dma_start(ident_sb[:], cident[:])
                fcos_sb = fpool.tile([128, TT, 64], f16)
                fsin_sb = fpool.tile([128, TT, 64], f16)
                wqkv_sb = wpool.tile([128, CT, 768], f16)
                wqkv_flat = wqkv_sb[:].rearrange("p a b -> p (a b)")
                # wqkv pieces ride gpsimd+scalar in ct consumption order,
                # leaving the sync channel exclusively for the x stream
                def wld(eng, c0, c1):
                    eng.dma_start(wqkv_flat[:, c0 * 768:c1 * 768],
                                  wqkv[:, c0 * 768:c1 * 768])
                wld(nc.gpsimd, 0, 1)
                wld(nc.scalar, 1, 2)
                wld(nc.gpsimd, 2, 3)
                wld(nc.scalar, 3, 4)
                wld(nc.gpsimd, 4, 6)
                wld(nc.scalar, 6, 8)
                wld(nc.gpsimd, 8, 10)
                wld(nc.scalar, 10, 12)
                wld(nc.gpsimd, 12, 14)
                wld(nc.scalar, 14, 16)
                nc.gpsimd.dma_start(fcos_sb[:].rearrange("p a b -> p (a b)"),
                                    fcos[:])
                nc.gpsimd.dma_start(fsin_sb[:].rearrange("p a b -> p (a b)"),
                                    fsin[:])
                nc.gpsimd.dma_start(ones_sb[:], cones[:])
                nc.gpsimd.dma_start(tri_sb[:], ctri[:])
                warm_act = cpool.tile([128, 1], f32)
                nc.scalar.activation(warm_act[:], tri_sb[:, 0:1],
                                     mybir.ActivationFunctionType.Exp,
                                     scale=1.0)
                # PE warm-up spin during startup DMAs (HAM needs ~3.4us busy);
                # memset source so warm-up needn't wait for the ident DMA
                for _ in range(36):
                    warm_ps = pptr.tile([128, 128], f16, tag="tr",
                                        name="warm_ps")
                    nc.tensor.transpose(warm_ps[:], wsrc[:], wsrc[:])

                pending_tr = []
                for ch in range(T // 256):  # 8 chunks of 256 t
                    scope = nc.named_scope(f"p1_ch{ch}")
                    scope.__enter__()
                    psq = [ppq.tile([128, 512], f32, tag="psq", name="psq")
                           for _ in range(2)]
                    pskv = [ppkv.tile([128, 256], f32, tag="pskv", name="pskv")
                            for _ in range(2)]
                    for cg in range(CT // 4):
                        xt = xtp.tile([128, 4, 256], f16, tag="xt")
                        col0 = (ch * 4 + cg) * 1024
                        xt_flat = xt[:].rearrange("p a b -> p (a b)")
                        if ch == 0 and cg == 0:
                            # ci-granular first loads so the first matmul
                            # waits on 64KB, not 256KB
                            for ci4 in range(4):
                                nc.sync.dma_start(
                                    xt_flat[:, ci4 * 256:(ci4 + 1) * 256],
                                    xT[:, col0 + ci4 * 256:
                                       col0 + (ci4 + 1) * 256],
                                )
                        else:
                            nc.sync.dma_start(xt_flat, xT[:, col0:col0 + 1024])
                        for ci in range(4):
                            ct = cg * 4 + ci
                            for t2 in range(2):
                                lhsT = xt[:, ci, t2 * 128:(t2 + 1) * 128]
                                nc.tensor.matmul(
                                    psq[t2][:], lhsT, wqkv_sb[:, ct, 0:512],
                                    start=(ct == 0), stop=(ct == CT - 1),
                                )
                                nc.tensor.matmul(
                                    pskv[t2][:], lhsT, wqkv_sb[:, ct, 512:768],
                                    start=(ct == 0), stop=(ct == CT - 1),
                                )
                    # rope (batched over the 4 q heads) — emitted now (DVE)
                    this_tr = []
                    for t2 in range(2):
                        tt = ch * 2 + t2
                        qr = qrp.tile([128, 512], f16, tag="qr")
                        kr = krp.tile([128, 128], f16, tag="kr")
                        cosb = bcast_mid(fcos_sb[:, tt, :], 4)
                        sinb = bcast_mid(fsin_sb[:, tt, :], 4)
                        qsrc = psq[t2][:].rearrange(
                            "p (h two j) -> p h two j", h=4, two=2
                        )
                        qdst = qr[:].rearrange(
                            "p (h two j) -> p h two j", h=4, two=2
                        )
                        te4, to4 = qsrc[:, :, 0, :], qsrc[:, :, 1, :]
                        a1 = rtp.tile([128, 4, 64], f16, tag="rt")
                        a2 = rtp.tile([128, 4, 64], f16, tag="rt")
                        nc.vector.tensor_mul(a1[:], te4, cosb)
                        nc.vector.tensor_mul(a2[:], to4, sinb)
                        nc.vector.tensor_sub(qdst[:, :, 0, :], a1[:], a2[:])
                        b1 = rtp.tile([128, 4, 64], f16, tag="rt")
                        b2 = rtp.tile([128, 4, 64], f16, tag="rt")
                        nc.vector.tensor_mul(b1[:], te4, sinb)
                        nc.vector.tensor_mul(b2[:], to4, cosb)
                        nc.vector.tensor_add(qdst[:, :, 1, :], b1[:], b2[:])
                        # K rope
                        kte, kto = pskv[t2][:, 0:64], pskv[t2][:, 64:128]
                        cos1 = fcos_sb[:, tt, :]
                        sin1 = fsin_sb[:, tt, :]
                        c1 = rtp.tile([128, 64], f16, tag="rtk")
                        c2 = rtp.tile([128, 64], f16, tag="rtk")
                        nc.vector.tensor_mul(c1[:], kte, cos1)
                        nc.vector.tensor_mul(c2[:], kto, sin1)
                        nc.vector.tensor_sub(kr[:, 0:64], c1[:], c2[:])
                        d1 = rtp.tile([128, 64], f16, tag="rtk")
                        d2 = rtp.tile([128, 64], f16, tag="rtk")
                        nc.vector.tensor_mul(d1[:], kte, sin1)
                        nc.vector.tensor_mul(d2[:], kto, cos1)
                        nc.vector.tensor_add(kr[:, 64:128], d1[:], d2[:])
                        nc.scalar.copy(v_sb[:, tt, :], pskv[t2][:, 128:256])
                        this_tr.append((tt, qr, kr))
                    # transposes for the PREVIOUS chunk (PE stays dense)
                    for tt, qr, kr in pending_tr:
                        for h in range(G):
                            ptr = pptr.tile([128, 128], f16, tag="tr",
                                            name="ptr")
                            nc.tensor.transpose(
                                ptr[:], qr[:, h * 128:(h + 1) * 128], ident_sb[:]
                            )
                            nc.scalar.copy(
                                qT_sb[:, h, tt * 128:(tt + 1) * 128], ptr[:]
                            )
                        ptr = pptr.tile([128, 128], f16, tag="tr", name="ptr")
                        nc.tensor.transpose(ptr[:], kr[:], ident_sb[:])
                        nc.scalar.copy(kT_sb[:, tt * 128:(tt + 1) * 128], ptr[:])
                    pending_tr = this_tr
                    scope.__exit__(None, None, None)
                for tt, qr, kr in pending_tr:
                    for h in range(G):
                        ptr = pptr.tile([128, 128], f16, tag="tr", name="ptr")
                        nc.tensor.transpose(
                            ptr[:], qr[:, h * 128:(h + 1) * 128], ident_sb[:]
                        )
                        nc.scalar.copy(
                            qT_sb[:, h, tt * 128:(tt + 1) * 128], ptr[:]
                        )
                    ptr = pptr.tile([128, 128], f16, tag="tr", name="ptr")
                    nc.tensor.transpose(ptr[:], kr[:], ident_sb[:])
                    nc.scalar.copy(kT_sb[:, tt * 128:(tt + 1) * 128], ptr[:])

            # ================= Phase 2+3: attention + output projection ======
            with (
                tc.tile_pool(name="wop", bufs=1) as wop,
                tc.tile_pool(name="expt", bufs=2) as expp,
                tc.tile_pool(name="ftmp", bufs=16) as ftp,
                tc.tile_pool(name="denb", bufs=1) as denp,
                tc.tile_pool(name="bcb", bufs=2) as bcp,
            ):
                wo_sb = wop.tile([128, G, C], f16)
                nc.sync.dma_start(wo_sb[:].rearrange("p a b -> p (a b)"), wo[:])

                def emit_wo_group(wopool, tc_j, t2, cc):
                    gt = tc_j * 4 + t2
                    psw = wopool.tile([128, 512], f32, tag="wo", name="psw")
                    for h in range(G):
                        nc.tensor.matmul(
                            psw[:],
                            outT_sb[:, h, gt * 128:(gt + 1) * 128],
                            wo_sb[:, h, cc * 512:(cc + 1) * 512],
                            start=(h == 0), stop=(h == G - 1),
                        )
                    # adjacent cc pairs share one [128,1024] store buffer:
                    # half the DMA issues and teardown waits
                    if cc % 2 == 0:
                        osb = opool.tile([128, 2, 512], f16, tag="osb",
                                         name="osb")
                        osb_hold[0] = osb
                    else:
                        osb = osb_hold[0]
                    half = cc % 2
                    # split the psum drain across both engines so the psw
                    # bank frees ~2x sooner
                    nc.scalar.copy(osb[:, half, 0:256], psw[:, 0:256])
                    nc.vector.tensor_copy(osb[:, half, 256:512],
                                          psw[:, 256:512])
                    if cc % 2 == 1:
                        store_eng = nc.gpsimd if (t2 * 2 + cc // 2) % 2 \
                            else nc.sync
                        store_eng.dma_start(
                            out[gt * 128:(gt + 1) * 128,
                                (cc - 1) * 512:(cc + 1) * 512],
                            osb[:].rearrange("p a b -> p (a b)"),
                        )

                osb_hold = [None]
                pending_wo = []

                psum_ctx = (
                    tc.tile_pool(name="pssc", bufs=4, space="PSUM"),
                    tc.tile_pool(name="psden", bufs=1, space="PSUM"),
                    tc.tile_pool(name="psav", bufs=1, space="PSUM"),
                    tc.tile_pool(name="pswo", bufs=2, space="PSUM"),
                )
                pssc = psum_ctx[0].__enter__()
                psden = psum_ctx[1].__enter__()
                psav = psum_ctx[2].__enter__()
                pswo = psum_ctx[3].__enter__()

                for tc_i in range(NTC):
                    scope = nc.named_scope(f"attn_tc{tc_i}")
                    scope.__enter__()
                    t0 = tc_i * 512
                    n_s = 4 * (tc_i + 1)
                    offs = [128 * (si - 4 * tc_i) if si >= 4 * tc_i else 0
                            for si in range(n_s)]
                    order = [4 * tc_i] + [si for si in range(n_s)
                                          if si != 4 * tc_i]
                    expTs = {}

                    def emit_scores(h):
                        expT = expp.tile([128, TT, 512], f16, tag="expT",
                                         name="expT")
                        expTs[h] = expT
                        for si in range(n_s):
                            off = offs[si]
                            ps = pssc.tile([128, 512], f32, tag="sc", name="ps")
                            nc.tensor.matmul(
                                ps[:, off:512],
                                kT_sb[:, si * 128:(si + 1) * 128],
                                qT_sb[:, h, t0 + off:t0 + 512],
                                start=True, stop=True,
                            )
                            if si >= 4 * tc_i:
                                nc.vector.tensor_add(
                                    ps[:, off:off + 128],
                                    ps[:, off:off + 128], tri_sb[:],
                                )
                            nc.scalar.activation(
                                expT[:, si, off:512], ps[:, off:512],
                                mybir.ActivationFunctionType.Exp, scale=SCALE,
                            )

                    bcs = {}

                    def emit_fold(h):
                        # DVE tree-fold of ALL exp tiles (fp16 2x mode) so
                        # the PE denominator pass is just 2 matmuls
                        expT = expTs[h]
                        nodes = [expT[:, si, :] for si in range(4 * tc_i + 1)]
                        while len(nodes) > 1:
                            nxt = []
                            for i in range(0, len(nodes) - 1, 2):
                                t = ftp.tile([128, 512], f16, tag="ft")
                                nc.vector.tensor_add(t[:], nodes[i],
                                                     nodes[i + 1])
                                nxt.append(t[:])
                            if len(nodes) % 2:
                                nxt.append(nodes[-1])
                            nodes = nxt
                        root = nodes[0]
                        # trimmed diagonal tiles, staggered valid ranges
                        d0 = 4 * tc_i
                        f = ftp.tile([128, 512], f16, tag="ft")
                        nc.vector.tensor_copy(f[:, 128:512],
                                              expT[:, d0 + 1, 128:512])
                        nc.vector.tensor_add(f[:, 256:512], f[:, 256:512],
                                             expT[:, d0 + 2, 256:512])
                        nc.vector.tensor_add(f[:, 384:512], f[:, 384:512],
                                             expT[:, d0 + 3, 384:512])
                        return root, f

                    def emit_den(h, es):
                        root, f = es
                        psd = psden.tile([1, 512], f32, tag="den", name="psd")
                        nc.tensor.matmul(psd[:, 0:512], ones_sb[:], root,
                                         start=True, stop=False)
                        nc.tensor.matmul(psd[:, 128:512], ones_sb[:],
                                         f[:, 128:512], start=False, stop=True)
                        den_r = denp.tile([1, 512], f32, tag="denr",
                                          name="den_r")
                        nc.vector.reciprocal_approx_fast(den_r[:], psd[:])
                        bc = bcp.tile([128, 512], f32, tag="bc", name="bc")
                        nc.gpsimd.partition_broadcast(bc[:], den_r[:])
                        bcs[h] = bc

                    def emit_av(h):
                        expT = expTs.pop(h)
                        bc = bcs.pop(h)
                        pso = psav.tile([128, 512], f32, tag="av", name="pso")
                        for i, si in enumerate(order):
                            off = offs[si]
                            nc.tensor.matmul(
                                pso[:, off:512], v_sb[:, si, :],
                                expT[:, si, off:512],
                                start=(i == 0), stop=(i == n_s - 1),
                            )
                        nc.vector.tensor_mul(
                            outT_sb[:, h, t0:t0 + 512], pso[:], bc[:]
                        )

                    # den/av split so the PE never waits on recip+broadcast:
                    # av(h) is emitted one slot after den(h), with other
                    # heads' matmuls covering the DVE/Pool latency.  The
                    # previous chunk's wo groups are drip-fed between steps
                    # so wo matmuls fill softmax-latency bubbles without
                    # their psum-drain copies queueing ahead of this
                    # chunk's exps on Scalar/Vector.
                    def drip(n):
                        for _ in range(n):
                            if pending_wo:
                                emit_wo_group(pswo, *pending_wo.pop(0))

                    emit_scores(0)
                    es0 = emit_fold(0)
                    emit_scores(1)
                    es1 = emit_fold(1)
                    emit_den(0, es0)
                    drip(2)
                    emit_scores(2)
                    es2 = emit_fold(2)
                    emit_av(0)
                    emit_den(1, es1)
                    drip(3)
                    emit_scores(3)
                    es3 = emit_fold(3)
                    emit_av(1)
                    emit_den(2, es2)
                    drip(3)
                    emit_av(2)
                    emit_den(3, es3)
                    drip(3)
                    emit_av(3)
                    drip(5)
                    scope.__exit__(None, None, None)

                    pending_wo = [(tc_i, t2, cc)
                                  for t2 in range(4) for cc in range(4)]

                for p in reversed(psum_ctx):
                    p.__exit__(None, None, None)

                scope = nc.named_scope("wo_tail")
                scope.__enter__()
                with tc.tile_pool(name="pswot", bufs=6,
                                  space="PSUM") as pswot:
                    for args in pending_wo:
                        emit_wo_group(pswot, *args)
                scope.__exit__(None, None, None)

    nc.finalize()
    return nc


def _prep_host(x, freqs_cos, freqs_sin, wq, wk, wv, wo):
    """Build per-core input maps."""
    x = np.asarray(x, dtype=np.float32)
    freqs_cos = np.asarray(freqs_cos, dtype=np.float32)
    freqs_sin = np.asarray(freqs_sin, dtype=np.float32)
    wq = np.asarray(wq, dtype=np.float32)
    wk = np.asarray(wk, dtype=np.float32)
    wv = np.asarray(wv, dtype=np.float32)
    wo = np.asarray(wo, dtype=np.float32)

    perm = np.concatenate([np.arange(0, HD, 2), np.arange(1, HD, 2)])
    # xT pre-tiled: [p, ch, cg, ci, t'] so each (ch, cg) load is contiguous
    xTs = []
    for b in range(B):
        A = np.ascontiguousarray(x[b].T)           # [C, T]
        A = A.reshape(4, 4, 128, 8, 256)           # [cg, ci, p, ch, t']
        A = A.transpose(2, 3, 0, 1, 4)             # [p, ch, cg, ci, t']
        xTs.append(np.ascontiguousarray(A.reshape(128, -1)).astype(np.float16))
    cident = np.eye(128, dtype=np.float16)
    cones = np.ones((128, 1), dtype=np.float16)
    ds, dt = np.meshgrid(np.arange(128), np.arange(128), indexing="ij")
    ctri = np.where(ds <= dt, 0.0, MASK_BIAS).astype(np.float32)

    in_maps = []
    for c in range(NCORES):
        b, kv = c // 4, c % 4
        cols = []
        for g in range(G):
            h = kv * G + g
            cols.append(wq[:, h * HD:(h + 1) * HD][:, perm])
        cols.append(wk[:, kv * HD:(kv + 1) * HD][:, perm])
        cols.append(wv[:, kv * HD:(kv + 1) * HD])
        wqkv_c = np.concatenate(cols, axis=1)              # [C, 768]
        wqkv_c = wqkv_c.reshape(CT, 128, 768).transpose(1, 0, 2)
        wqkv_c = np.ascontiguousarray(wqkv_c.reshape(128, -1)).astype(np.float16)
        wo_c = wo[kv * G * HD:(kv + 1) * G * HD, :]        # [512, C]
        wo_c = wo_c.reshape(G, 128, C).transpose(1, 0, 2)
        wo_c = np.ascontiguousarray(wo_c.reshape(128, -1)).astype(np.float16)
        fc = np.ascontiguousarray(
            freqs_cos.reshape(TT, 128, 64).transpose(1, 0, 2).reshape(128, -1)).astype(np.float16)
        fs = np.ascontiguousarray(
            freqs_sin.reshape(TT, 128, 64).transpose(1, 0, 2).reshape(128, -1)).astype(np.float16)
        in_maps.append({
            "xT": xTs[b],
            "wqkv": wqkv_c,
            "wo": wo_c,
            "fcos": fc,
            "fsin": fs,
            "cident": cident,
            "cones": cones,
            "ctri": ctri,
        })
    return in_maps


def _install_ntff_hook_shim():
    """bass_utils trace=True needs antenv.axon_hooks, absent in this image.
    Provide it in sys.modules and register the ctypes NTFF hook."""
    import types

    if "antenv.axon_hooks" in sys.modules:
        return
    mod = types.ModuleType("antenv.axon_hooks")
    mod._hook = None
    mod.set_axon_ntff_profile_hook = lambda h: setattr(mod, "_hook", h)
    mod.get_axon_ntff_profile_hook = lambda: mod._hook
    sys.modules["antenv.axon_hooks"] = mod
    try:
        from trn_agent_boot.trn_boot import _ntff_profile_via_ctypes

        mod._hook = _ntff_profile_via_ctypes("/opt/axon/libaxon_pjrt.so")
    except Exception:
        pass


def kernel(x, freqs_cos, freqs_sin, wq, wk, wv, wo, trace=False):
    global LAST_RESULTS
    from concourse.bass_utils import run_bass_kernel_spmd

    if trace:
        _install_ntff_hook_shim()

    if "nc" not in _CACHE:
        _CACHE["nc"] = _build()
    nc = _CACHE["nc"]

    in_maps = _prep_host(x, freqs_cos, freqs_sin, wq, wk, wv, wo)
    res = run_bass_kernel_spmd(nc, in_maps, core_ids=list(range(NCORES)),
                               trace=trace)
    LAST_RESULTS = res
    out = np.zeros((B, T, C), dtype=np.float32)
    for c in range(NCORES):
        out[c // 4] += res.results[c]["out"].astype(np.float32)
    return out

